# revision 43
# baseline (speedup 1.0000x reference)
"""Llama decode block (single token) on 8 TRN2 NeuronCores, tensor-parallel.

Sharding (per sharding_hint): w_q/w_k/w_v/w_ff1 column-sharded, w_o/w_ff2
row-sharded, KV cache sharded by head (4 heads/core). AllReduce after the
attention output projection and after w_ff2. The residual stream x is folded
into the all-reduces as x/8 per core, so each AR output is the full residual
sum directly.

Dtype strategy (validated numerically against the rel-err metric):
  - attention weights + KV cache in fp16 (e5m10): 2 B/elem halves DMA bytes,
    matmuls run at full PE rate; error lands ~1e-2 on the metric (gate 2e-2)
  - FFN weights/activations as bf16 hi+lo splits (w = hi + lo, three matmul
    passes hi*hi + hi*lo + lo*hi accumulated in one PSUM group): fp32-like
    accuracy at full bf16 PE rate; bytes same as f32
  - residual stream, softmax, rmsnorm, collectives in f32

Per-core dataflow:
  rmsnorm(x) -> h16 cols [128,32] fp16
  qT[128d,4h] via weight-stationary matmuls (wq tiles stationary, h moving)
  scores on PE: host-transposed kcT tiles stationary, qT cols moving,
  per 1024-token super-tile: 32 score matmuls -> one exp -> 32 AV matmuls
  k/v rows for the current token; RoPE; appended analytically
  o @ w_o + x/8 -> AllReduce #1 -> x2
  rmsnorm(x2) -> h2 hi/lo; silu(h2 @ w_ff1) hi/lo -> a hi/lo
  a @ w_ff2 + x2/8 -> AllReduce #2 -> y
"""

import math

import numpy as np

import concourse.bass as bass
import concourse.mybir as mybir
import concourse.tile as tile
from concourse import bacc
from concourse import bass_utils

F32 = mybir.dt.float32
F16 = mybir.dt.float16
BF16 = mybir.dt.bfloat16
F8E5 = mybir.dt.float8e5
AF = mybir.ActivationFunctionType
ALU = mybir.AluOpType

HIDDEN = 4096
N_HEADS = 32
HEAD_DIM = 128
INTERM = 11008
KV_LEN = 4096
N_CORES = 8

HEADS_PC = N_HEADS // N_CORES          # 4 heads per core
QKV_N = HEADS_PC * HEAD_DIM            # 512
FF_N = INTERM // N_CORES               # 1376
KB = HIDDEN // 128                     # 32 k-blocks of the hidden dim
FF_KB_SIZES = [128] * 10 + [96]        # 1376 = 10*128 + 96
LO_SCALE = 4096.0                      # fp8e5m2 lo-term scaling (2^12)
SCALE = 1.0 / math.sqrt(HEAD_DIM)


def _emit(nc, tc):
    i = {}

    def din(name, shape, dt=F32):
        i[name] = nc.dram_tensor(name, list(shape), dt, kind="ExternalInput").ap()

    din("x", [HIDDEN])
    din("attn_norm", [HIDDEN])
    din("ffn_norm", [HIDDEN])
    din("sin", [HEAD_DIM // 2])
    din("cos", [HEAD_DIM // 2])
    din("ident32", [32, 32])
    din("wqkv", [HIDDEN, 3 * QKV_N], F16)
    din("kc", [KV_LEN, QKV_N], F16)
    din("vc", [KV_LEN, QKV_N], F16)
    din("wo", [QKV_N, HIDDEN], F16)
    din("wf1h", [HIDDEN, FF_N], F16)
    din("wf1l", [HIDDEN, FF_N], F8E5)
    din("wf2h", [FF_N, HIDDEN], F16)
    din("wf2l", [FF_N, HIDDEN], F8E5)
    y = nc.dram_tensor("y", [HIDDEN], F32, kind="ExternalOutput").ap()

    with (
        tc.tile_pool(name="const", bufs=1) as cpool,
        tc.tile_pool(name="wkvp", bufs=2) as wkvp,
        tc.tile_pool(name="kp", bufs=2) as kp,
        tc.tile_pool(name="vp", bufs=2) as vp,
        tc.tile_pool(name="wop", bufs=2) as wop,
        tc.tile_pool(name="wf1p", bufs=3) as wf1p,
        tc.tile_pool(name="wf1lp", bufs=3) as wf1lp,
        tc.tile_pool(name="wf2p", bufs=2) as wf2p,
        tc.tile_pool(name="wf2lp", bufs=2) as wf2lp,
        tc.tile_pool(name="sm", bufs=1) as sm,
        tc.tile_pool(name="psum", bufs=8, space="PSUM") as pp,
        tc.tile_pool(name="dram", bufs=1, space="DRAM") as dram,
    ):
        # ---- constants ----
        ones32 = cpool.tile([32, 1], F32)
        nc.vector.memset(ones32[:], 1.0)
        ones128h = cpool.tile([128, 1], F16)
        nc.vector.memset(ones128h[:], 1.0)
        ones_r32 = cpool.tile([1, 32], F32)
        nc.vector.memset(ones_r32[:], 1.0)
        ones_r128 = cpool.tile([1, 128], F32)
        nc.vector.memset(ones_r128[:], 1.0)
        eighth = cpool.tile([1, 1], F32)
        nc.vector.memset(eighth[:], 1.0 / N_CORES)
        eps11 = cpool.tile([1, 1], F32)
        nc.vector.memset(eps11[:], 1e-6)
        ident1 = cpool.tile([1, 1], F32)
        nc.vector.memset(ident1[:], 1.0)
        ident1h = cpool.tile([1, 1], F16)
        nc.vector.memset(ident1h[:], 1.0)
        ident8 = cpool.tile([1, 1], F8E5)
        nc.vector.memset(ident8[:], 1.0)
        ident32 = cpool.tile([32, 32], F32)
        nc.sync.dma_start(ident32[:], i["ident32"])

        # sin/cos as rows (for k RoPE) and columns (for qT RoPE, q-scaled)
        sin_row = cpool.tile([1, 64], F32)
        cos_row = cpool.tile([1, 64], F32)
        nc.sync.dma_start(sin_row[:], i["sin"].rearrange("(a d) -> a d", a=1))
        nc.sync.dma_start(cos_row[:], i["cos"].rearrange("(a d) -> a d", a=1))
        sinq_row = cpool.tile([1, 64], F32)
        cosq_row = cpool.tile([1, 64], F32)
        nc.vector.tensor_scalar_mul(sinq_row[:], sin_row[:], SCALE)
        nc.vector.tensor_scalar_mul(cosq_row[:], cos_row[:], SCALE)

        # ---- rmsnorm -> cols [128, 32] f32 ----
        def rmsnorm_cols(x_dram, norm_dram, tag):
            x_rows = sm.tile([32, 128], F32, name=f"x_rows_{tag}", tag="x_rows")
            nrm_rows = sm.tile([32, 128], F32, name=f"nrm_rows_{tag}", tag="nrm_rows")
            nc.sync.dma_start(x_rows[:], x_dram.rearrange("(a d) -> a d", a=32))
            nc.sync.dma_start(nrm_rows[:], norm_dram.rearrange("(a d) -> a d", a=32))
            sq = sm.tile([32, 128], F32, name=f"sq_{tag}", tag="sq")
            ssq = sm.tile([32, 1], F32, name=f"ssq_{tag}", tag="ssq")
            nc.scalar.activation(sq[:], x_rows[:], AF.Square, accum_out=ssq[:])
            ms_psum = pp.tile([1, 1], F32, name=f"ms_psum_{tag}", tag="ps")
            nc.tensor.matmul(ms_psum[:], ones32[:], ssq[:])
            rstd = sm.tile([1, 1], F32, name=f"rstd_{tag}", tag="rstd")
            nc.scalar.activation(rstd[:], ms_psum[:], AF.Sqrt,
                                 bias=eps11[:], scale=1.0 / HIDDEN)
            nc.vector.reciprocal(rstd[:], rstd[:])
            rstd_ps = pp.tile([32, 1], F32, name=f"rstd_ps_{tag}", tag="ps")
            nc.tensor.matmul(rstd_ps[:], ones_r32[:], rstd[:])
            rstd32 = sm.tile([32, 1], F32, name=f"rstd32_{tag}", tag="rstd32")
            nc.vector.tensor_copy(rstd32[:], rstd_ps[:])
            h_rows = sm.tile([32, 128], F32, name=f"h_rows_{tag}", tag="h_rows")
            nc.vector.tensor_tensor(h_rows[:], x_rows[:], nrm_rows[:], ALU.mult)
            nc.vector.tensor_scalar_mul(h_rows[:], h_rows[:], rstd32[:])
            h_psum = pp.tile([128, 32], F32, name=f"h_psum_{tag}", tag="ps")
            nc.tensor.transpose(h_psum[:], h_rows[:], ident32[:])
            h_cols = sm.tile([128, 32], F32, name=f"h_cols_{tag}", tag="hcols")
            nc.vector.tensor_copy(h_cols[:], h_psum[:])
            return h_cols

        h_cols = rmsnorm_cols(i["x"], i["attn_norm"], "a")
        h16 = sm.tile([128, 32], F16, name="h16")
        nc.vector.tensor_copy(h16[:], h_cols[:])

        # ---- fused q/k/v rows: h @ [wq|wk|wv], weights moving ----
        q_ps = pp.tile([1, QKV_N], F32, name="q_ps", tag="ps")
        k_ps = pp.tile([1, QKV_N], F32, name="k_ps", tag="ps")
        v_ps = pp.tile([1, QKV_N], F32, name="v_ps", tag="ps")
        for t8 in range(16):
            wkv_t = wkvp.tile([128, 2, 3 * QKV_N], F16, name="wkv_t", tag="wkv")
            nc.sync.dma_start(
                wkv_t[:],
                i["wqkv"][t8 * 256:(t8 + 1) * 256, :].rearrange(
                    "(b p) c -> p b c", p=128),
            )
            for b in range(2):
                kb = t8 * 2 + b
                nc.tensor.matmul(q_ps[:], h16[:, kb:kb + 1],
                                 wkv_t[:, b, 0:QKV_N],
                                 start=(kb == 0), stop=(kb == KB - 1))
                nc.tensor.matmul(k_ps[:], h16[:, kb:kb + 1],
                                 wkv_t[:, b, QKV_N:2 * QKV_N],
                                 start=(kb == 0), stop=(kb == KB - 1))
                nc.tensor.matmul(v_ps[:], h16[:, kb:kb + 1],
                                 wkv_t[:, b, 2 * QKV_N:3 * QKV_N],
                                 start=(kb == 0), stop=(kb == KB - 1))

        def rope_row(src_ps, cos_t, sin_t, tag):
            out = sm.tile([1, QKV_N], F32, name=f"rope_{tag}")
            tmp = sm.tile([1, QKV_N], F32, name=f"rope_tmp_{tag}")
            r3 = src_ps[:].rearrange("a (h d) -> a h d", h=HEADS_PC)
            o3 = out[:].rearrange("a (h d) -> a h d", h=HEADS_PC)
            t3 = tmp[:].rearrange("a (h d) -> a h d", h=HEADS_PC)
            cb = cos_t[:].unsqueeze(1).to_broadcast((1, HEADS_PC, 64))
            sb = sin_t[:].unsqueeze(1).to_broadcast((1, HEADS_PC, 64))
            nc.vector.tensor_tensor(o3[:, :, 0:64], r3[:, :, 0:64], cb, ALU.mult)
            nc.vector.tensor_tensor(t3[:, :, 0:64], r3[:, :, 64:128], sb, ALU.mult)
            nc.vector.tensor_sub(o3[:, :, 0:64], o3[:, :, 0:64], t3[:, :, 0:64])
            nc.vector.tensor_tensor(o3[:, :, 64:128], r3[:, :, 64:128], cb, ALU.mult)
            nc.vector.tensor_tensor(t3[:, :, 64:128], r3[:, :, 0:64], sb, ALU.mult)
            nc.vector.tensor_add(o3[:, :, 64:128], o3[:, :, 64:128],
                                 t3[:, :, 64:128])
            return out

        q_rot = rope_row(q_ps, cosq_row, sinq_row, "q")  # pre-scaled
        qrep_ps = pp.tile([128, QKV_N], F32, name="qrep_ps", tag="ps")
        nc.tensor.matmul(qrep_ps[:], ones_r128[:], q_rot[:])
        q_rep16 = sm.tile([128, QKV_N], F16, name="q_rep16")
        nc.vector.tensor_copy(q_rep16[:], qrep_ps[:])

        # ---- attention: DVE scores; AV via 97-col stationary so the 4
        # heads' outputs land on aligned PSUM rows 0/32/64/96 ----
        oacc = pp.tile([97, 512], F32, name="oacc", tag="ps")
        den_acc = sm.tile([1, HEADS_PC], F32, name="den_acc")
        nc.vector.memset(den_acc[:], 0.0)

        for g in range(4):
            k_sup = kp.tile([128, 8, QKV_N], F16, name="k_sup", tag="k")
            nc.sync.dma_start(
                k_sup[:],
                i["kc"][g * 1024:(g + 1) * 1024, :].rearrange(
                    "(b p) c -> p b c", p=128),
            )
            v_sup = vp.tile([128, 8, QKV_N], F16, name="v_sup", tag="v")
            nc.sync.dma_start(
                v_sup[:],
                i["vc"][g * 1024:(g + 1) * 1024, :].rearrange(
                    "(b p) c -> p b c", p=128),
            )
            s_f32 = sm.tile([128, 32], F32, name=f"s_f32_{g}", tag=f"sf{g % 2}")
            qb = q_rep16[:].unsqueeze(1).to_broadcast((128, 4, QKV_N))
            for half in range(2):
                scratch = sm.tile([128, 4, QKV_N], F32, name=f"scr_{g}_{half}",
                                  tag="scr")
                nc.vector.tensor_tensor(scratch[:],
                                        k_sup[:, half * 4:(half + 1) * 4, :],
                                        qb, ALU.mult)
                nc.vector.tensor_reduce(
                    s_f32[:, half * 16:(half + 1) * 16].rearrange(
                        "p (t h) -> p t h", h=HEADS_PC),
                    scratch[:].rearrange("p t (h d) -> p t h d", h=HEADS_PC),
                    mybir.AxisListType.X, ALU.add)
            exp_c = sm.tile([128, 32], F16, name=f"exp_{g}", tag=f"exp{g % 2}")
            nc.scalar.activation(exp_c[:], s_f32[:], AF.Exp)
            s_av = sm.tile([128, 8, 128], F16, name=f"s_av_{g}",
                           tag=f"sav{g % 2}")
            nc.vector.memset(s_av[:], 0.0)
            sav_view = s_av[:].rearrange("p t (h j) -> p t h j", j=32)
            nc.vector.tensor_copy(
                sav_view[:, :, :, 0:1],
                exp_c[:].rearrange("p (t h j) -> p t h j", h=HEADS_PC, j=1))
            den_ps = pp.tile([1, 32], F32, name="den_ps", tag="ps")
            nc.tensor.matmul(den_ps[:], ones128h[:], exp_c[:])
            den_g = sm.tile([1, HEADS_PC], F32, name="den_g", tag="deng")
            nc.vector.tensor_reduce(
                den_g[:],
                den_ps[:].rearrange("a (t h) -> a h t", h=HEADS_PC),
                mybir.AxisListType.X, ALU.add)
            nc.vector.tensor_add(den_acc[:], den_acc[:], den_g[:])
            for tt in range(8):
                nc.tensor.matmul(
                    oacc[:],
                    s_av[:, tt, 0:97],
                    v_sup[:, tt, :],
                    start=(g == 0 and tt == 0),
                    stop=(g == 3 and tt == 7),
                    skip_group_check=True,
                )

        # ---- current-token contribution (on rows) ----
        k_rot = rope_row(k_ps, cos_row, sin_row, "k")  # unscaled
        v16_row = sm.tile([1, QKV_N], F16, name="v16_row")
        nc.vector.tensor_copy(v16_row[:], v_ps[:])

        scr_new = sm.tile([1, QKV_N], F32, name="scr_new")
        nc.vector.tensor_tensor(scr_new[:], q_rot[:], k_rot[:], ALU.mult)
        s_new = sm.tile([1, HEADS_PC], F32, name="s_new")
        nc.vector.tensor_reduce(
            s_new[:],
            scr_new[:].rearrange("a (h d) -> a h d", h=HEADS_PC),
            mybir.AxisListType.X, ALU.add)
        e_new = sm.tile([1, HEADS_PC], F32, name="e_new")
        nc.scalar.activation(e_new[:], s_new[:], AF.Exp)
        nc.vector.tensor_add(den_acc[:], den_acc[:], e_new[:])

        # o row = (sum_t exp*v + e_new*v_new) / den, then transpose to cols
        o_row = sm.tile([1, QKV_N], F32, name="o_row_att")
        o3v = o_row[:].rearrange("a (h d) -> a h d", h=HEADS_PC)
        for h in range(HEADS_PC):
            nc.vector.tensor_copy(o_row[:, h * 128:(h + 1) * 128],
                                  oacc[32 * h:32 * h + 1,
                                       h * 128:(h + 1) * 128])
        vnew_sc = sm.tile([1, QKV_N], F32, name="vnew_sc")
        v3 = vnew_sc[:].rearrange("a (h d) -> a h d", h=HEADS_PC)
        eb = e_new[:].unsqueeze(2).to_broadcast((1, HEADS_PC, 128))
        nc.vector.tensor_tensor(v3[:], v_ps[:].rearrange(
            "a (h d) -> a h d", h=HEADS_PC), eb, ALU.mult)
        nc.vector.tensor_add(o_row[:], o_row[:], vnew_sc[:])
        nc.vector.reciprocal(den_acc[:], den_acc[:])
        rb = den_acc[:].unsqueeze(2).to_broadcast((1, HEADS_PC, 128))
        nc.vector.tensor_tensor(o3v[:], o3v[:], rb, ALU.mult)

        oT_ps = pp.tile([128, HEADS_PC], F32, name="oT_ps", tag="ps")
        for h in range(HEADS_PC):
            nc.tensor.transpose(oT_ps[:, h:h + 1],
                                o_row[:, h * 128:(h + 1) * 128], ident1[:])
        o_sb = sm.tile([128, HEADS_PC], F16, name="o_sb")
        nc.vector.tensor_copy(o_sb[:], oT_ps[:])

        # ---- o @ w_o + x/8 -> [1,4096] -> AllReduce #1 ----
        ar1_in = dram.tile([HIDDEN], F32, name="ar1_in")
        ar1_out = dram.tile([HIDDEN], F32, name="ar1_out")

        chunks1 = [pp.tile([1, 512], F32, name=f"c1_{n}", tag="ps")
                   for n in range(8)]
        for kb in range(HEADS_PC):
            wo_t = wop.tile([128, HIDDEN], F16, name="wo_t", tag="wo")
            nc.sync.dma_start(wo_t[:], i["wo"][kb * 128:(kb + 1) * 128, :])
            for n in range(8):
                nc.tensor.matmul(
                    chunks1[n][:], o_sb[:, kb:kb + 1],
                    wo_t[:, n * 512:(n + 1) * 512],
                    start=(kb == 0), stop=False,
                )
        for n in range(8):
            xch = sm.tile([1, 512], F32, name=f"xr_{n}", tag=f"xr{n % 2}")
            nc.sync.dma_start(
                xch[:], i["x"][n * 512:(n + 1) * 512].rearrange("(a d) -> a d", a=1))
            nc.tensor.matmul(
                chunks1[n][:], eighth[:], xch[:],
                start=False, stop=True,
            )
            orow_c = sm.tile([1, 512], F32, name=f"or_{n}", tag=f"or{n % 2}")
            nc.vector.tensor_copy(orow_c[:], chunks1[n][:])
            nc.sync.dma_start(ar1_in[n * 512:(n + 1) * 512], orow_c[:])
        nc.gpsimd.collective_compute(
            "AllReduce", ALU.add,
            replica_groups=[list(range(N_CORES))],
            ins=[ar1_in[:].opt()], outs=[ar1_out[:].opt()],
        )

        # ---- MLP ----
        h2_cols = rmsnorm_cols(ar1_out[:], i["ffn_norm"], "b")

        # h2 hi/lo fp16; s1[kb] = [h2h | 0*31 | h2l] stationaries (M=33);
        # h2hs8 = fp8e5m2(h2h / LO_SCALE) pairs with the scaled fp8 lo weights
        h2h = sm.tile([128, 32], F16, name="h2h")
        nc.vector.tensor_copy(h2h[:], h2_cols[:])
        h2h32 = sm.tile([128, 32], F32, name="h2h32")
        nc.vector.tensor_copy(h2h32[:], h2h[:])
        h2hs8 = sm.tile([128, 32], F8E5, name="h2hs8")
        nc.vector.tensor_scalar_mul(h2hs8[:], h2h32[:], 1.0 / LO_SCALE)
        nc.vector.tensor_sub(h2h32[:], h2_cols[:], h2h32[:])
        s1 = sm.tile([128, 32, 33], F16, name="s1")
        nc.vector.memset(s1[:], 0.0)
        h2c3 = h2h[:].rearrange("p (k j) -> p k j", j=1)
        l2c3 = h2h32[:].rearrange("p (k j) -> p k j", j=1)
        nc.vector.tensor_copy(s1[:, :, 0:1], h2c3)
        nc.vector.tensor_copy(s1[:, :, 32:33], l2c3)

        # wf1: h2-stationary (M=33: hi-part row 0, lo-part row 32), w moving
        FF1_CH = [(0, 512), (512, 1024), (1024, 1376)]
        pre_ps = [pp.tile([33, c1 - c0], F32, name=f"pre_{ci}", tag="ps")
                  for ci, (c0, c1) in enumerate(FF1_CH)]
        for t8 in range(8):
            w1h_t = wf1p.tile([128, 4, FF_N], F16, name="w1h_t", tag="wf1")
            nc.sync.dma_start(
                w1h_t[:],
                i["wf1h"][t8 * 512:(t8 + 1) * 512, :].rearrange(
                    "(b p) c -> p b c", p=128),
            )
            w1l_t = wf1lp.tile([128, 4, FF_N], F8E5, name="w1l_t", tag="wf1l")
            nc.sync.dma_start(
                w1l_t[:],
                i["wf1l"][t8 * 512:(t8 + 1) * 512, :].rearrange(
                    "(b p) c -> p b c", p=128),
            )
            for b in range(4):
                kb = t8 * 4 + b
                for ci, (c0, c1) in enumerate(FF1_CH):
                    nc.tensor.matmul(
                        pre_ps[ci][:],
                        s1[:, kb, :],
                        w1h_t[:, b, c0:c1],
                        start=(kb == 0), stop=False,
                        skip_group_check=True,
                    )
                    nc.tensor.matmul(
                        pre_ps[ci][0:1, :],
                        h2hs8[:, kb:kb + 1],
                        w1l_t[:, b, c0:c1],
                        start=False, stop=(kb == KB - 1),
                        skip_group_check=True,
                    )

        # pre = row0 + row32; silu on the row; a -> hi/lo rows
        pre_row = sm.tile([1, FF_N], F32, name="pre_row")
        for ci, (c0, c1) in enumerate(FF1_CH):
            pc = sm.tile([1, 512], F32, name=f"pc_{ci}", tag=f"pc{ci % 2}")
            nc.vector.tensor_copy(pc[:, 0:c1 - c0], pre_ps[ci][32:33, :])
            nc.vector.tensor_copy(pre_row[:, c0:c1], pre_ps[ci][0:1, :])
            nc.vector.tensor_tensor(pre_row[:, c0:c1], pre_row[:, c0:c1],
                                    pc[:, 0:c1 - c0], ALU.add)
        sig_row = sm.tile([1, FF_N], F32, name="sig_row", tag="row32a")
        nc.scalar.activation(sig_row[:], pre_row[:], AF.Sigmoid)
        a_row = pre_row  # in-place: a = pre * sigmoid(pre)
        nc.vector.tensor_tensor(a_row[:], pre_row[:], sig_row[:], ALU.mult)
        ah_row = sm.tile([1, FF_N], F16, name="ah_row")
        nc.vector.tensor_copy(ah_row[:], a_row[:])
        ah32_row = sm.tile([1, FF_N], F32, name="ah32_row", tag="row32b")
        nc.vector.tensor_copy(ah32_row[:], ah_row[:])
        aS_row = sm.tile([1, FF_N], F8E5, name="aS_row")
        nc.vector.tensor_scalar_mul(aS_row[:], ah32_row[:], 1.0 / LO_SCALE)
        nc.vector.tensor_sub(ah32_row[:], a_row[:], ah32_row[:])
        al_row = sm.tile([1, FF_N], F16, name="al_row")
        nc.vector.tensor_copy(al_row[:], ah32_row[:])

        # transpose a rows to columns (even cols: 4B-aligned PSUM writes);
        # build s2[kb] = [a_hi | 0*31 | a_lo] fp16 and s2s = fp8 scaled-hi
        aT_ps = pp.tile([128, 44], F16, name="aT_ps", tag="ps")
        aTs_ps = pp.tile([128, 44], F8E5, name="aTs_ps", tag="ps")
        for kb in range(11):
            sz = FF_KB_SIZES[kb]
            nc.tensor.transpose(aT_ps[0:sz, 2 * kb:2 * kb + 1],
                                ah_row[:, kb * 128:kb * 128 + sz], ident1h[:])
            nc.tensor.transpose(aT_ps[0:sz, 22 + 2 * kb:23 + 2 * kb],
                                al_row[:, kb * 128:kb * 128 + sz], ident1h[:])
            nc.tensor.transpose(aTs_ps[0:sz, 4 * kb:4 * kb + 1],
                                aS_row[:, kb * 128:kb * 128 + sz], ident8[:])
        s2 = sm.tile([128, 11, 33], F16, name="s2")
        nc.vector.memset(s2[:], 0.0)
        aTh3 = aT_ps[:, 0:22].rearrange("p (k j) -> p k j", j=2)
        aTl3 = aT_ps[:, 22:44].rearrange("p (k j) -> p k j", j=2)
        nc.vector.tensor_copy(s2[:, :, 0:1], aTh3[:, :, 0:1])
        nc.vector.tensor_copy(s2[:, :, 32:33], aTl3[:, :, 0:1])
        s2s = sm.tile([128, 11], F8E5, name="s2s")
        aTs3 = aTs_ps[:].rearrange("p (k j) -> p k j", j=4)
        s2s3 = s2s[:].rearrange("p (k j) -> p k j", j=1)
        nc.vector.tensor_copy(s2s3, aTs3[:, :, 0:1])

        # wf2: a-stationary (M=33), weights moving, two passes
        chunks2 = [pp.tile([33, 512], F32, name=f"c2_{n}", tag="ps")
                   for n in range(8)]
        for kb in range(11):
            sz = FF_KB_SIZES[kb]
            w2h_t = wf2p.tile([128, HIDDEN], F16, name="w2h_t", tag="wf2")
            nc.sync.dma_start(
                w2h_t[0:sz, :], i["wf2h"][kb * 128:kb * 128 + sz, :])
            w2l_t = wf2lp.tile([128, HIDDEN], F8E5, name="w2l_t", tag="wf2l")
            nc.sync.dma_start(
                w2l_t[0:sz, :], i["wf2l"][kb * 128:kb * 128 + sz, :])
            for n in range(8):
                nc.tensor.matmul(
                    chunks2[n][:],
                    s2[0:sz, kb, :],
                    w2h_t[0:sz, n * 512:(n + 1) * 512],
                    start=(kb == 0), stop=False,
                    skip_group_check=True,
                )
                nc.tensor.matmul(
                    chunks2[n][0:1, :],
                    s2s[0:sz, kb:kb + 1],
                    w2l_t[0:sz, n * 512:(n + 1) * 512],
                    start=False, stop=False,
                    skip_group_check=True,
                )

        ar2_in = dram.tile([HIDDEN], F32, name="ar2_in")
        ar2_out = dram.tile([HIDDEN], F32, name="ar2_out")
        for n in range(8):
            x2ch = sm.tile([1, 512], F32, name=f"x2r_{n}", tag=f"xr{n % 2}")
            nc.sync.dma_start(
                x2ch[:],
                ar1_out[n * 512:(n + 1) * 512].rearrange("(a d) -> a d", a=1))
            nc.tensor.matmul(
                chunks2[n][0:1, :], eighth[:], x2ch[:],
                start=False, stop=True,
                skip_group_check=True,
            )
            c2sb = sm.tile([1, 512], F32, name=f"c2sb_{n}", tag=f"pc{n % 2}")
            nc.vector.tensor_copy(c2sb[:], chunks2[n][32:33, :])
            ffc = sm.tile([1, 512], F32, name=f"ff_{n}", tag=f"or{n % 2}")
            nc.vector.tensor_copy(ffc[:], chunks2[n][0:1, :])
            nc.vector.tensor_tensor(ffc[:], ffc[:], c2sb[:], ALU.add)
            nc.sync.dma_start(ar2_in[n * 512:(n + 1) * 512], ffc[:])
        nc.gpsimd.collective_compute(
            "AllReduce", ALU.add,
            replica_groups=[list(range(N_CORES))],
            ins=[ar2_in[:].opt()], outs=[ar2_out[:].opt()],
        )
        nc.sync.dma_start(y[:], ar2_out[:])


_BUILT = None


def _build():
    global _BUILT
    if _BUILT is None:
        nc = bacc.Bacc("TRN2", target_bir_lowering=False, debug=False,
                       num_devices=N_CORES)
        with tile.TileContext(nc) as tc:
            _emit(nc, tc)
        nc.compile()
        _BUILT = nc
    return _BUILT


def _shard(inputs):
    import ml_dtypes
    E5 = ml_dtypes.float8_e5m2

    f = lambda a: np.ascontiguousarray(np.asarray(a, dtype=np.float32))
    f16 = lambda a: np.ascontiguousarray(np.asarray(a, dtype=np.float16))

    def hilo(a):
        hi = np.asarray(a, dtype=np.float16)
        lo = np.asarray((a - hi.astype(np.float32)) * 4096.0, dtype=E5)
        return np.ascontiguousarray(hi), np.ascontiguousarray(lo)

    x = f(inputs["x"])
    attn_norm = f(inputs["attn_norm"])
    ffn_norm = f(inputs["ffn_norm"])
    pos = int(np.asarray(inputs["pos"]))
    sin = f(inputs["sin_cache"][pos])
    cos = f(inputs["cos_cache"][pos])
    wq, wk, wv = [np.asarray(inputs[k], np.float32) for k in ("w_q", "w_k", "w_v")]
    wo = np.asarray(inputs["w_o"], np.float32)
    wf1 = np.asarray(inputs["w_ff1"], np.float32)
    wf2 = np.asarray(inputs["w_ff2"], np.float32)
    kc = np.asarray(inputs["k_cache"], np.float32).reshape(KV_LEN, N_HEADS * HEAD_DIM)
    vc = np.asarray(inputs["v_cache"], np.float32).reshape(KV_LEN, N_HEADS * HEAD_DIM)

    in_maps = []
    for c in range(N_CORES):
        qs = slice(c * QKV_N, (c + 1) * QKV_N)
        fs = slice(c * FF_N, (c + 1) * FF_N)
        w1h, w1l = hilo(wf1[:, fs])
        w2h, w2l = hilo(wf2[fs, :])
        in_maps.append({
            "x": x,
            "ident32": np.eye(32, dtype=np.float32),
            "attn_norm": attn_norm,
            "ffn_norm": ffn_norm,
            "sin": sin,
            "cos": cos,
            "wqkv": f16(np.concatenate([wq[:, qs], wk[:, qs], wv[:, qs]], axis=1)),
            "kc": f16(kc[:, qs]),
            "vc": f16(vc[:, qs]),
            "wo": f16(wo[qs, :]),
            "wf1h": w1h,
            "wf1l": w1l,
            "wf2h": w2h,
            "wf2l": w2l,
        })
    return in_maps


def kernel(**inputs):
    nc = _build()
    in_maps = _shard(inputs)
    res = bass_utils.run_bass_kernel_spmd(
        nc, in_maps, core_ids=list(range(N_CORES)))
    return res.results[0]["y"]


# revision 44
# speedup vs baseline: 1.0845x; 1.0845x over previous
"""Llama decode block (single token) on 8 TRN2 NeuronCores, tensor-parallel.

Sharding (per sharding_hint): w_q/w_k/w_v/w_ff1 column-sharded, w_o/w_ff2
row-sharded, KV cache sharded by head (4 heads/core). AllReduce after the
attention output projection and after w_ff2. The residual stream x is folded
into the all-reduces as x/8 per core, so each AR output is the full residual
sum directly.

Dtype strategy (validated numerically against the rel-err metric):
  - attention weights + KV cache in fp16 (e5m10): 2 B/elem halves DMA bytes,
    matmuls run at full PE rate; error lands ~1e-2 on the metric (gate 2e-2)
  - FFN weights/activations as bf16 hi+lo splits (w = hi + lo, three matmul
    passes hi*hi + hi*lo + lo*hi accumulated in one PSUM group): fp32-like
    accuracy at full bf16 PE rate; bytes same as f32
  - residual stream, softmax, rmsnorm, collectives in f32

Per-core dataflow:
  rmsnorm(x) -> h16 cols [128,32] fp16
  qT[128d,4h] via weight-stationary matmuls (wq tiles stationary, h moving)
  scores on PE: host-transposed kcT tiles stationary, qT cols moving,
  per 1024-token super-tile: 32 score matmuls -> one exp -> 32 AV matmuls
  k/v rows for the current token; RoPE; appended analytically
  o @ w_o + x/8 -> AllReduce #1 -> x2
  rmsnorm(x2) -> h2 hi/lo; silu(h2 @ w_ff1) hi/lo -> a hi/lo
  a @ w_ff2 + x2/8 -> AllReduce #2 -> y
"""

import math

import numpy as np

import concourse.bass as bass
import concourse.mybir as mybir
import concourse.tile as tile
from concourse import bacc
from concourse import bass_utils

F32 = mybir.dt.float32
F16 = mybir.dt.float16
BF16 = mybir.dt.bfloat16
F8E5 = mybir.dt.float8e5
AF = mybir.ActivationFunctionType
ALU = mybir.AluOpType

HIDDEN = 4096
N_HEADS = 32
HEAD_DIM = 128
INTERM = 11008
KV_LEN = 4096
N_CORES = 8

HEADS_PC = N_HEADS // N_CORES          # 4 heads per core
QKV_N = HEADS_PC * HEAD_DIM            # 512
FF_N = INTERM // N_CORES               # 1376
KB = HIDDEN // 128                     # 32 k-blocks of the hidden dim
FF_KB_SIZES = [128] * 10 + [96]        # 1376 = 10*128 + 96
LO_SCALE = 4096.0                      # fp8e5m2 lo-term scaling (2^12)
SCALE = 1.0 / math.sqrt(HEAD_DIM)


def _emit(nc, tc):
    i = {}

    def din(name, shape, dt=F32):
        i[name] = nc.dram_tensor(name, list(shape), dt, kind="ExternalInput").ap()

    din("x", [HIDDEN])
    din("attn_norm", [HIDDEN])
    din("ffn_norm", [HIDDEN])
    din("sin", [HEAD_DIM // 2])
    din("cos", [HEAD_DIM // 2])
    din("ident32", [32, 32])
    din("wqkv", [16, 128, 2, 3 * QKV_N], F16)
    din("kc", [4, 128, 8, QKV_N], F16)
    din("vc", [4, 128, 8, QKV_N], F16)
    din("wo", [QKV_N, HIDDEN], F16)
    din("wf1h", [8, 128, 4, FF_N], F16)
    din("wf1l", [8, 128, 4, FF_N], F8E5)
    din("wf2h", [FF_N, HIDDEN], F16)
    din("wf2l", [FF_N, HIDDEN], F8E5)
    y = nc.dram_tensor("y", [HIDDEN], F32, kind="ExternalOutput").ap()

    with (
        tc.tile_pool(name="const", bufs=1) as cpool,
        tc.tile_pool(name="wkvp", bufs=2) as wkvp,
        tc.tile_pool(name="kp", bufs=2) as kp,
        tc.tile_pool(name="vp", bufs=2) as vp,
        tc.tile_pool(name="wop", bufs=2) as wop,
        tc.tile_pool(name="wf1p", bufs=3) as wf1p,
        tc.tile_pool(name="wf1lp", bufs=3) as wf1lp,
        tc.tile_pool(name="wf2p", bufs=2) as wf2p,
        tc.tile_pool(name="wf2lp", bufs=2) as wf2lp,
        tc.tile_pool(name="sm", bufs=1) as sm,
        tc.tile_pool(name="psum", bufs=8, space="PSUM") as pp,
        tc.tile_pool(name="dram", bufs=1, space="DRAM") as dram,
    ):
        # ---- constants ----
        ones32 = cpool.tile([32, 1], F32)
        nc.vector.memset(ones32[:], 1.0)
        ones128h = cpool.tile([128, 1], F16)
        nc.vector.memset(ones128h[:], 1.0)
        ones_r32 = cpool.tile([1, 32], F32)
        nc.vector.memset(ones_r32[:], 1.0)
        ones_r128 = cpool.tile([1, 128], F32)
        nc.vector.memset(ones_r128[:], 1.0)
        eighth = cpool.tile([1, 1], F32)
        nc.vector.memset(eighth[:], 1.0 / N_CORES)
        eps11 = cpool.tile([1, 1], F32)
        nc.vector.memset(eps11[:], 1e-6)
        ident1 = cpool.tile([1, 1], F32)
        nc.vector.memset(ident1[:], 1.0)
        ident1h = cpool.tile([1, 1], F16)
        nc.vector.memset(ident1h[:], 1.0)
        ident8 = cpool.tile([1, 1], F8E5)
        nc.vector.memset(ident8[:], 1.0)
        ident32 = cpool.tile([32, 32], F32)
        nc.sync.dma_start(ident32[:], i["ident32"])

        # sin/cos as rows (for k RoPE) and columns (for qT RoPE, q-scaled)
        sin_row = cpool.tile([1, 64], F32)
        cos_row = cpool.tile([1, 64], F32)
        nc.sync.dma_start(sin_row[:], i["sin"].rearrange("(a d) -> a d", a=1))
        nc.sync.dma_start(cos_row[:], i["cos"].rearrange("(a d) -> a d", a=1))
        sinq_row = cpool.tile([1, 64], F32)
        cosq_row = cpool.tile([1, 64], F32)
        nc.vector.tensor_scalar_mul(sinq_row[:], sin_row[:], SCALE)
        nc.vector.tensor_scalar_mul(cosq_row[:], cos_row[:], SCALE)

        # ---- rmsnorm -> cols [128, 32] f32 ----
        def rmsnorm_cols(x_dram, norm_dram, tag):
            x_rows = sm.tile([32, 128], F32, name=f"x_rows_{tag}", tag="x_rows")
            nrm_rows = sm.tile([32, 128], F32, name=f"nrm_rows_{tag}", tag="nrm_rows")
            nc.sync.dma_start(x_rows[:], x_dram.rearrange("(a d) -> a d", a=32))
            nc.sync.dma_start(nrm_rows[:], norm_dram.rearrange("(a d) -> a d", a=32))
            sq = sm.tile([32, 128], F32, name=f"sq_{tag}", tag="sq")
            ssq = sm.tile([32, 1], F32, name=f"ssq_{tag}", tag="ssq")
            nc.scalar.activation(sq[:], x_rows[:], AF.Square, accum_out=ssq[:])
            ms_psum = pp.tile([1, 1], F32, name=f"ms_psum_{tag}", tag="ps")
            nc.tensor.matmul(ms_psum[:], ones32[:], ssq[:])
            rstd = sm.tile([1, 1], F32, name=f"rstd_{tag}", tag="rstd")
            nc.scalar.activation(rstd[:], ms_psum[:], AF.Sqrt,
                                 bias=eps11[:], scale=1.0 / HIDDEN)
            nc.vector.reciprocal(rstd[:], rstd[:])
            rstd_ps = pp.tile([32, 1], F32, name=f"rstd_ps_{tag}", tag="ps")
            nc.tensor.matmul(rstd_ps[:], ones_r32[:], rstd[:])
            rstd32 = sm.tile([32, 1], F32, name=f"rstd32_{tag}", tag="rstd32")
            nc.vector.tensor_copy(rstd32[:], rstd_ps[:])
            h_rows = sm.tile([32, 128], F32, name=f"h_rows_{tag}", tag="h_rows")
            nc.vector.tensor_tensor(h_rows[:], x_rows[:], nrm_rows[:], ALU.mult)
            nc.vector.tensor_scalar_mul(h_rows[:], h_rows[:], rstd32[:])
            h_psum = pp.tile([128, 32], F32, name=f"h_psum_{tag}", tag="ps")
            nc.tensor.transpose(h_psum[:], h_rows[:], ident32[:])
            h_cols = sm.tile([128, 32], F32, name=f"h_cols_{tag}", tag="hcols")
            nc.vector.tensor_copy(h_cols[:], h_psum[:])
            return h_cols

        h_cols = rmsnorm_cols(i["x"], i["attn_norm"], "a")
        h16 = sm.tile([128, 32], F16, name="h16")
        nc.vector.tensor_copy(h16[:], h_cols[:])

        # ---- fused q/k/v rows: h @ [wq|wk|wv], weights moving ----
        q_ps = pp.tile([1, QKV_N], F32, name="q_ps", tag="ps")
        k_ps = pp.tile([1, QKV_N], F32, name="k_ps", tag="ps")
        v_ps = pp.tile([1, QKV_N], F32, name="v_ps", tag="ps")
        for t8 in range(16):
            wkv_t = wkvp.tile([128, 2, 3 * QKV_N], F16, name="wkv_t", tag="wkv")
            nc.sync.dma_start(wkv_t[:], i["wqkv"][t8])
            for b in range(2):
                kb = t8 * 2 + b
                nc.tensor.matmul(q_ps[:], h16[:, kb:kb + 1],
                                 wkv_t[:, b, 0:QKV_N],
                                 start=(kb == 0), stop=(kb == KB - 1))
                nc.tensor.matmul(k_ps[:], h16[:, kb:kb + 1],
                                 wkv_t[:, b, QKV_N:2 * QKV_N],
                                 start=(kb == 0), stop=(kb == KB - 1))
                nc.tensor.matmul(v_ps[:], h16[:, kb:kb + 1],
                                 wkv_t[:, b, 2 * QKV_N:3 * QKV_N],
                                 start=(kb == 0), stop=(kb == KB - 1))

        def rope_row(src_ps, cos_t, sin_t, tag):
            out = sm.tile([1, QKV_N], F32, name=f"rope_{tag}")
            tmp = sm.tile([1, QKV_N], F32, name=f"rope_tmp_{tag}")
            r3 = src_ps[:].rearrange("a (h d) -> a h d", h=HEADS_PC)
            o3 = out[:].rearrange("a (h d) -> a h d", h=HEADS_PC)
            t3 = tmp[:].rearrange("a (h d) -> a h d", h=HEADS_PC)
            cb = cos_t[:].unsqueeze(1).to_broadcast((1, HEADS_PC, 64))
            sb = sin_t[:].unsqueeze(1).to_broadcast((1, HEADS_PC, 64))
            nc.vector.tensor_tensor(o3[:, :, 0:64], r3[:, :, 0:64], cb, ALU.mult)
            nc.vector.tensor_tensor(t3[:, :, 0:64], r3[:, :, 64:128], sb, ALU.mult)
            nc.vector.tensor_sub(o3[:, :, 0:64], o3[:, :, 0:64], t3[:, :, 0:64])
            nc.vector.tensor_tensor(o3[:, :, 64:128], r3[:, :, 64:128], cb, ALU.mult)
            nc.vector.tensor_tensor(t3[:, :, 64:128], r3[:, :, 0:64], sb, ALU.mult)
            nc.vector.tensor_add(o3[:, :, 64:128], o3[:, :, 64:128],
                                 t3[:, :, 64:128])
            return out

        q_rot = rope_row(q_ps, cosq_row, sinq_row, "q")  # pre-scaled
        qrep_ps = pp.tile([128, QKV_N], F32, name="qrep_ps", tag="ps")
        nc.tensor.matmul(qrep_ps[:], ones_r128[:], q_rot[:])
        q_rep16 = sm.tile([128, QKV_N], F16, name="q_rep16")
        nc.vector.tensor_copy(q_rep16[:], qrep_ps[:])

        # ---- attention: DVE scores; AV via 97-col stationary so the 4
        # heads' outputs land on aligned PSUM rows 0/32/64/96 ----
        oacc = pp.tile([97, 512], F32, name="oacc", tag="ps")
        den_acc = sm.tile([1, HEADS_PC], F32, name="den_acc")
        nc.vector.memset(den_acc[:], 0.0)

        for g in range(4):
            k_sup = kp.tile([128, 8, QKV_N], F16, name="k_sup", tag="k")
            nc.sync.dma_start(k_sup[:], i["kc"][g])
            v_sup = vp.tile([128, 8, QKV_N], F16, name="v_sup", tag="v")
            nc.sync.dma_start(v_sup[:], i["vc"][g])
            s_f32 = sm.tile([128, 32], F32, name=f"s_f32_{g}", tag=f"sf{g % 2}")
            qb = q_rep16[:].unsqueeze(1).to_broadcast((128, 4, QKV_N))
            for half in range(2):
                scratch = sm.tile([128, 4, QKV_N], F32, name=f"scr_{g}_{half}",
                                  tag="scr")
                nc.vector.tensor_tensor(scratch[:],
                                        k_sup[:, half * 4:(half + 1) * 4, :],
                                        qb, ALU.mult)
                nc.vector.tensor_reduce(
                    s_f32[:, half * 16:(half + 1) * 16].rearrange(
                        "p (t h) -> p t h", h=HEADS_PC),
                    scratch[:].rearrange("p t (h d) -> p t h d", h=HEADS_PC),
                    mybir.AxisListType.X, ALU.add)
            exp_c = sm.tile([128, 32], F16, name=f"exp_{g}", tag=f"exp{g % 2}")
            nc.scalar.activation(exp_c[:], s_f32[:], AF.Exp)
            s_av = sm.tile([128, 8, 128], F16, name=f"s_av_{g}",
                           tag=f"sav{g % 2}")
            nc.vector.memset(s_av[:], 0.0)
            sav_view = s_av[:].rearrange("p t (h j) -> p t h j", j=32)
            nc.vector.tensor_copy(
                sav_view[:, :, :, 0:1],
                exp_c[:].rearrange("p (t h j) -> p t h j", h=HEADS_PC, j=1))
            den_ps = pp.tile([1, 32], F32, name="den_ps", tag="ps")
            nc.tensor.matmul(den_ps[:], ones128h[:], exp_c[:])
            den_g = sm.tile([1, HEADS_PC], F32, name="den_g", tag="deng")
            nc.vector.tensor_reduce(
                den_g[:],
                den_ps[:].rearrange("a (t h) -> a h t", h=HEADS_PC),
                mybir.AxisListType.X, ALU.add)
            nc.vector.tensor_add(den_acc[:], den_acc[:], den_g[:])
            for tt in range(8):
                nc.tensor.matmul(
                    oacc[:],
                    s_av[:, tt, 0:97],
                    v_sup[:, tt, :],
                    start=(g == 0 and tt == 0),
                    stop=(g == 3 and tt == 7),
                    skip_group_check=True,
                )

        # ---- current-token contribution (on rows) ----
        k_rot = rope_row(k_ps, cos_row, sin_row, "k")  # unscaled
        v16_row = sm.tile([1, QKV_N], F16, name="v16_row")
        nc.vector.tensor_copy(v16_row[:], v_ps[:])

        scr_new = sm.tile([1, QKV_N], F32, name="scr_new")
        nc.vector.tensor_tensor(scr_new[:], q_rot[:], k_rot[:], ALU.mult)
        s_new = sm.tile([1, HEADS_PC], F32, name="s_new")
        nc.vector.tensor_reduce(
            s_new[:],
            scr_new[:].rearrange("a (h d) -> a h d", h=HEADS_PC),
            mybir.AxisListType.X, ALU.add)
        e_new = sm.tile([1, HEADS_PC], F32, name="e_new")
        nc.scalar.activation(e_new[:], s_new[:], AF.Exp)
        nc.vector.tensor_add(den_acc[:], den_acc[:], e_new[:])

        # o row = (sum_t exp*v + e_new*v_new) / den, then transpose to cols
        o_row = sm.tile([1, QKV_N], F32, name="o_row_att")
        o3v = o_row[:].rearrange("a (h d) -> a h d", h=HEADS_PC)
        for h in range(HEADS_PC):
            nc.vector.tensor_copy(o_row[:, h * 128:(h + 1) * 128],
                                  oacc[32 * h:32 * h + 1,
                                       h * 128:(h + 1) * 128])
        vnew_sc = sm.tile([1, QKV_N], F32, name="vnew_sc")
        v3 = vnew_sc[:].rearrange("a (h d) -> a h d", h=HEADS_PC)
        eb = e_new[:].unsqueeze(2).to_broadcast((1, HEADS_PC, 128))
        nc.vector.tensor_tensor(v3[:], v_ps[:].rearrange(
            "a (h d) -> a h d", h=HEADS_PC), eb, ALU.mult)
        nc.vector.tensor_add(o_row[:], o_row[:], vnew_sc[:])
        nc.vector.reciprocal(den_acc[:], den_acc[:])
        rb = den_acc[:].unsqueeze(2).to_broadcast((1, HEADS_PC, 128))
        nc.vector.tensor_tensor(o3v[:], o3v[:], rb, ALU.mult)

        oT_ps = pp.tile([128, HEADS_PC], F32, name="oT_ps", tag="ps")
        for h in range(HEADS_PC):
            nc.tensor.transpose(oT_ps[:, h:h + 1],
                                o_row[:, h * 128:(h + 1) * 128], ident1[:])
        o_sb = sm.tile([128, HEADS_PC], F16, name="o_sb")
        nc.vector.tensor_copy(o_sb[:], oT_ps[:])

        # ---- o @ w_o + x/8 -> [1,4096] -> AllReduce #1 ----
        ar1_in = dram.tile([HIDDEN], F32, name="ar1_in")
        ar1_out = dram.tile([HIDDEN], F32, name="ar1_out")

        chunks1 = [pp.tile([1, 512], F32, name=f"c1_{n}", tag="ps")
                   for n in range(8)]
        for kb in range(HEADS_PC):
            wo_t = wop.tile([128, HIDDEN], F16, name="wo_t", tag="wo")
            nc.sync.dma_start(wo_t[:], i["wo"][kb * 128:(kb + 1) * 128, :])
            for n in range(8):
                nc.tensor.matmul(
                    chunks1[n][:], o_sb[:, kb:kb + 1],
                    wo_t[:, n * 512:(n + 1) * 512],
                    start=(kb == 0), stop=False,
                )
        for n in range(8):
            xch = sm.tile([1, 512], F32, name=f"xr_{n}", tag=f"xr{n % 2}")
            nc.sync.dma_start(
                xch[:], i["x"][n * 512:(n + 1) * 512].rearrange("(a d) -> a d", a=1))
            nc.tensor.matmul(
                chunks1[n][:], eighth[:], xch[:],
                start=False, stop=True,
            )
            orow_c = sm.tile([1, 512], F32, name=f"or_{n}", tag=f"or{n % 2}")
            nc.vector.tensor_copy(orow_c[:], chunks1[n][:])
            nc.sync.dma_start(ar1_in[n * 512:(n + 1) * 512], orow_c[:])
        nc.gpsimd.collective_compute(
            "AllReduce", ALU.add,
            replica_groups=[list(range(N_CORES))],
            ins=[ar1_in[:].opt()], outs=[ar1_out[:].opt()],
        )

        # ---- MLP ----
        h2_cols = rmsnorm_cols(ar1_out[:], i["ffn_norm"], "b")

        # h2 hi/lo fp16; s1[kb] = [h2h | 0*31 | h2l] stationaries (M=33);
        # h2hs8 = fp8e5m2(h2h / LO_SCALE) pairs with the scaled fp8 lo weights
        h2h = sm.tile([128, 32], F16, name="h2h")
        nc.vector.tensor_copy(h2h[:], h2_cols[:])
        h2h32 = sm.tile([128, 32], F32, name="h2h32")
        nc.vector.tensor_copy(h2h32[:], h2h[:])
        h2hs8 = sm.tile([128, 32], F8E5, name="h2hs8")
        nc.vector.tensor_scalar_mul(h2hs8[:], h2h32[:], 1.0 / LO_SCALE)
        nc.vector.tensor_sub(h2h32[:], h2_cols[:], h2h32[:])
        s1 = sm.tile([128, 32, 33], F16, name="s1")
        nc.vector.memset(s1[:], 0.0)
        h2c3 = h2h[:].rearrange("p (k j) -> p k j", j=1)
        l2c3 = h2h32[:].rearrange("p (k j) -> p k j", j=1)
        nc.vector.tensor_copy(s1[:, :, 0:1], h2c3)
        nc.vector.tensor_copy(s1[:, :, 32:33], l2c3)

        # wf1: h2-stationary (M=33: hi-part row 0, lo-part row 32), w moving
        FF1_CH = [(0, 512), (512, 1024), (1024, 1376)]
        pre_ps = [pp.tile([33, c1 - c0], F32, name=f"pre_{ci}", tag="ps")
                  for ci, (c0, c1) in enumerate(FF1_CH)]
        for t8 in range(8):
            w1h_t = wf1p.tile([128, 4, FF_N], F16, name="w1h_t", tag="wf1")
            nc.sync.dma_start(w1h_t[:], i["wf1h"][t8])
            w1l_t = wf1lp.tile([128, 4, FF_N], F8E5, name="w1l_t", tag="wf1l")
            nc.sync.dma_start(w1l_t[:], i["wf1l"][t8])
            for b in range(4):
                kb = t8 * 4 + b
                for ci, (c0, c1) in enumerate(FF1_CH):
                    nc.tensor.matmul(
                        pre_ps[ci][:],
                        s1[:, kb, :],
                        w1h_t[:, b, c0:c1],
                        start=(kb == 0), stop=False,
                        skip_group_check=True,
                    )
                    nc.tensor.matmul(
                        pre_ps[ci][0:1, :],
                        h2hs8[:, kb:kb + 1],
                        w1l_t[:, b, c0:c1],
                        start=False, stop=(kb == KB - 1),
                        skip_group_check=True,
                    )

        # pre = row0 + row32; silu on the row; a -> hi/lo rows
        pre_row = sm.tile([1, FF_N], F32, name="pre_row")
        for ci, (c0, c1) in enumerate(FF1_CH):
            pc = sm.tile([1, 512], F32, name=f"pc_{ci}", tag=f"pc{ci % 2}")
            nc.vector.tensor_copy(pc[:, 0:c1 - c0], pre_ps[ci][32:33, :])
            nc.vector.tensor_copy(pre_row[:, c0:c1], pre_ps[ci][0:1, :])
            nc.vector.tensor_tensor(pre_row[:, c0:c1], pre_row[:, c0:c1],
                                    pc[:, 0:c1 - c0], ALU.add)
        sig_row = sm.tile([1, FF_N], F32, name="sig_row", tag="row32a")
        nc.scalar.activation(sig_row[:], pre_row[:], AF.Sigmoid)
        a_row = pre_row  # in-place: a = pre * sigmoid(pre)
        nc.vector.tensor_tensor(a_row[:], pre_row[:], sig_row[:], ALU.mult)
        ah_row = sm.tile([1, FF_N], F16, name="ah_row")
        nc.vector.tensor_copy(ah_row[:], a_row[:])
        ah32_row = sm.tile([1, FF_N], F32, name="ah32_row", tag="row32b")
        nc.vector.tensor_copy(ah32_row[:], ah_row[:])
        aS_row = sm.tile([1, FF_N], F8E5, name="aS_row")
        nc.vector.tensor_scalar_mul(aS_row[:], ah32_row[:], 1.0 / LO_SCALE)
        nc.vector.tensor_sub(ah32_row[:], a_row[:], ah32_row[:])
        al_row = sm.tile([1, FF_N], F16, name="al_row")
        nc.vector.tensor_copy(al_row[:], ah32_row[:])

        # transpose a rows to columns (even cols: 4B-aligned PSUM writes);
        # build s2[kb] = [a_hi | 0*31 | a_lo] fp16 and s2s = fp8 scaled-hi
        aT_ps = pp.tile([128, 44], F16, name="aT_ps", tag="ps")
        aTs_ps = pp.tile([128, 44], F8E5, name="aTs_ps", tag="ps")
        for kb in range(11):
            sz = FF_KB_SIZES[kb]
            nc.tensor.transpose(aT_ps[0:sz, 2 * kb:2 * kb + 1],
                                ah_row[:, kb * 128:kb * 128 + sz], ident1h[:])
            nc.tensor.transpose(aT_ps[0:sz, 22 + 2 * kb:23 + 2 * kb],
                                al_row[:, kb * 128:kb * 128 + sz], ident1h[:])
            nc.tensor.transpose(aTs_ps[0:sz, 4 * kb:4 * kb + 1],
                                aS_row[:, kb * 128:kb * 128 + sz], ident8[:])
        s2 = sm.tile([128, 11, 33], F16, name="s2")
        nc.vector.memset(s2[:], 0.0)
        aTh3 = aT_ps[:, 0:22].rearrange("p (k j) -> p k j", j=2)
        aTl3 = aT_ps[:, 22:44].rearrange("p (k j) -> p k j", j=2)
        nc.vector.tensor_copy(s2[:, :, 0:1], aTh3[:, :, 0:1])
        nc.vector.tensor_copy(s2[:, :, 32:33], aTl3[:, :, 0:1])
        s2s = sm.tile([128, 11], F8E5, name="s2s")
        aTs3 = aTs_ps[:].rearrange("p (k j) -> p k j", j=4)
        s2s3 = s2s[:].rearrange("p (k j) -> p k j", j=1)
        nc.vector.tensor_copy(s2s3, aTs3[:, :, 0:1])

        # wf2: a-stationary (M=33), weights moving, two passes
        chunks2 = [pp.tile([33, 512], F32, name=f"c2_{n}", tag="ps")
                   for n in range(8)]
        for kb in range(11):
            sz = FF_KB_SIZES[kb]
            w2h_t = wf2p.tile([128, HIDDEN], F16, name="w2h_t", tag="wf2")
            nc.sync.dma_start(
                w2h_t[0:sz, :], i["wf2h"][kb * 128:kb * 128 + sz, :])
            w2l_t = wf2lp.tile([128, HIDDEN], F8E5, name="w2l_t", tag="wf2l")
            nc.sync.dma_start(
                w2l_t[0:sz, :], i["wf2l"][kb * 128:kb * 128 + sz, :])
            for n in range(8):
                nc.tensor.matmul(
                    chunks2[n][:],
                    s2[0:sz, kb, :],
                    w2h_t[0:sz, n * 512:(n + 1) * 512],
                    start=(kb == 0), stop=False,
                    skip_group_check=True,
                )
                nc.tensor.matmul(
                    chunks2[n][0:1, :],
                    s2s[0:sz, kb:kb + 1],
                    w2l_t[0:sz, n * 512:(n + 1) * 512],
                    start=False, stop=False,
                    skip_group_check=True,
                )

        ar2_in = dram.tile([HIDDEN], F32, name="ar2_in")
        ar2_out = dram.tile([HIDDEN], F32, name="ar2_out")
        for n in range(8):
            x2ch = sm.tile([1, 512], F32, name=f"x2r_{n}", tag=f"xr{n % 2}")
            nc.sync.dma_start(
                x2ch[:],
                ar1_out[n * 512:(n + 1) * 512].rearrange("(a d) -> a d", a=1))
            nc.tensor.matmul(
                chunks2[n][0:1, :], eighth[:], x2ch[:],
                start=False, stop=True,
                skip_group_check=True,
            )
            c2sb = sm.tile([1, 512], F32, name=f"c2sb_{n}", tag=f"pc{n % 2}")
            nc.vector.tensor_copy(c2sb[:], chunks2[n][32:33, :])
            ffc = sm.tile([1, 512], F32, name=f"ff_{n}", tag=f"or{n % 2}")
            nc.vector.tensor_copy(ffc[:], chunks2[n][0:1, :])
            nc.vector.tensor_tensor(ffc[:], ffc[:], c2sb[:], ALU.add)
            nc.sync.dma_start(ar2_in[n * 512:(n + 1) * 512], ffc[:])
        nc.gpsimd.collective_compute(
            "AllReduce", ALU.add,
            replica_groups=[list(range(N_CORES))],
            ins=[ar2_in[:].opt()], outs=[ar2_out[:].opt()],
        )
        nc.sync.dma_start(y[:], ar2_out[:])


_BUILT = None


def _build():
    global _BUILT
    if _BUILT is None:
        nc = bacc.Bacc("TRN2", target_bir_lowering=False, debug=False,
                       num_devices=N_CORES)
        with tile.TileContext(nc) as tc:
            _emit(nc, tc)
        nc.compile()
        _BUILT = nc
    return _BUILT


def _shard(inputs):
    import ml_dtypes
    E5 = ml_dtypes.float8_e5m2

    f = lambda a: np.ascontiguousarray(np.asarray(a, dtype=np.float32))
    f16 = lambda a: np.ascontiguousarray(np.asarray(a, dtype=np.float16))

    def hilo(a):
        hi = np.asarray(a, dtype=np.float16)
        lo = np.asarray((a - hi.astype(np.float32)) * 4096.0, dtype=E5)
        return np.ascontiguousarray(hi), np.ascontiguousarray(lo)

    x = f(inputs["x"])
    attn_norm = f(inputs["attn_norm"])
    ffn_norm = f(inputs["ffn_norm"])
    pos = int(np.asarray(inputs["pos"]))
    sin = f(inputs["sin_cache"][pos])
    cos = f(inputs["cos_cache"][pos])
    wq, wk, wv = [np.asarray(inputs[k], np.float32) for k in ("w_q", "w_k", "w_v")]
    wo = np.asarray(inputs["w_o"], np.float32)
    wf1 = np.asarray(inputs["w_ff1"], np.float32)
    wf2 = np.asarray(inputs["w_ff2"], np.float32)
    kc = np.asarray(inputs["k_cache"], np.float32).reshape(KV_LEN, N_HEADS * HEAD_DIM)
    vc = np.asarray(inputs["v_cache"], np.float32).reshape(KV_LEN, N_HEADS * HEAD_DIM)

    in_maps = []
    for c in range(N_CORES):
        qs = slice(c * QKV_N, (c + 1) * QKV_N)
        fs = slice(c * FF_N, (c + 1) * FF_N)
        w1h, w1l = hilo(wf1[:, fs])
        w2h, w2l = hilo(wf2[fs, :])
        tile4 = lambda a, nb, b: np.ascontiguousarray(
            a.reshape(nb, b, 128, a.shape[1]).transpose(0, 2, 1, 3))
        in_maps.append({
            "x": x,
            "ident32": np.eye(32, dtype=np.float32),
            "attn_norm": attn_norm,
            "ffn_norm": ffn_norm,
            "sin": sin,
            "cos": cos,
            "wqkv": tile4(f16(np.concatenate(
                [wq[:, qs], wk[:, qs], wv[:, qs]], axis=1)), 16, 2),
            "kc": tile4(f16(kc[:, qs]), 4, 8),
            "vc": tile4(f16(vc[:, qs]), 4, 8),
            "wo": f16(wo[qs, :]),
            "wf1h": tile4(w1h, 8, 4),
            "wf1l": tile4(w1l, 8, 4),
            "wf2h": w2h,
            "wf2l": w2l,
        })
    return in_maps


def kernel(**inputs):
    nc = _build()
    in_maps = _shard(inputs)
    res = bass_utils.run_bass_kernel_spmd(
        nc, in_maps, core_ids=list(range(N_CORES)))
    return res.results[0]["y"]


# revision 45
# speedup vs baseline: 1.1641x; 1.0733x over previous
"""Llama decode block (single token) on 8 TRN2 NeuronCores, tensor-parallel.

Sharding (per sharding_hint): w_q/w_k/w_v/w_ff1 column-sharded, w_o/w_ff2
row-sharded, KV cache sharded by head (4 heads/core). AllReduce after the
attention output projection and after w_ff2. The residual stream x is folded
into the all-reduces as x/8 per core, so each AR output is the full residual
sum directly.

Dtype strategy (validated numerically against the rel-err metric):
  - attention weights + KV cache in fp16 (e5m10): 2 B/elem halves DMA bytes,
    matmuls run at full PE rate; error lands ~1e-2 on the metric (gate 2e-2)
  - FFN weights/activations as bf16 hi+lo splits (w = hi + lo, three matmul
    passes hi*hi + hi*lo + lo*hi accumulated in one PSUM group): fp32-like
    accuracy at full bf16 PE rate; bytes same as f32
  - residual stream, softmax, rmsnorm, collectives in f32

Per-core dataflow:
  rmsnorm(x) -> h16 cols [128,32] fp16
  qT[128d,4h] via weight-stationary matmuls (wq tiles stationary, h moving)
  scores on PE: host-transposed kcT tiles stationary, qT cols moving,
  per 1024-token super-tile: 32 score matmuls -> one exp -> 32 AV matmuls
  k/v rows for the current token; RoPE; appended analytically
  o @ w_o + x/8 -> AllReduce #1 -> x2
  rmsnorm(x2) -> h2 hi/lo; silu(h2 @ w_ff1) hi/lo -> a hi/lo
  a @ w_ff2 + x2/8 -> AllReduce #2 -> y
"""

import math

import numpy as np

import concourse.bass as bass
import concourse.mybir as mybir
import concourse.tile as tile
from concourse import bacc
from concourse import bass_utils

F32 = mybir.dt.float32
F16 = mybir.dt.float16
BF16 = mybir.dt.bfloat16
F8E5 = mybir.dt.float8e5
AF = mybir.ActivationFunctionType
ALU = mybir.AluOpType

HIDDEN = 4096
N_HEADS = 32
HEAD_DIM = 128
INTERM = 11008
KV_LEN = 4096
N_CORES = 8

HEADS_PC = N_HEADS // N_CORES          # 4 heads per core
QKV_N = HEADS_PC * HEAD_DIM            # 512
FF_N = INTERM // N_CORES               # 1376
KB = HIDDEN // 128                     # 32 k-blocks of the hidden dim
FF_KB_SIZES = [128] * 10 + [96]        # 1376 = 10*128 + 96
LO_SCALE = 4096.0                      # fp8e5m2 lo-term scaling (2^12)
SCALE = 1.0 / math.sqrt(HEAD_DIM)


def _emit(nc, tc):
    i = {}

    def din(name, shape, dt=F32):
        i[name] = nc.dram_tensor(name, list(shape), dt, kind="ExternalInput").ap()

    din("x", [HIDDEN])
    din("attn_norm", [HIDDEN])
    din("ffn_norm", [HIDDEN])
    din("sin", [HEAD_DIM // 2])
    din("cos", [HEAD_DIM // 2])
    din("ident32", [32, 32])
    din("wqkv", [8, 128, 4, 3 * QKV_N], F16)
    din("kc", [4, 128, 8, QKV_N], F16)
    din("vc", [4, 128, 8, QKV_N], F16)
    din("wo", [QKV_N, HIDDEN], F16)
    din("wf1h", [8, 128, 4, FF_N], F16)
    din("wf1l", [8, 128, 4, FF_N], F8E5)
    din("wf2h", [FF_N, HIDDEN], F16)
    din("wf2l", [FF_N, HIDDEN], F8E5)
    y = nc.dram_tensor("y", [HIDDEN], F32, kind="ExternalOutput").ap()

    with (
        tc.tile_pool(name="const", bufs=1) as cpool,
        tc.tile_pool(name="wkvp", bufs=2) as wkvp,
        tc.tile_pool(name="kp", bufs=2) as kp,
        tc.tile_pool(name="vp", bufs=2) as vp,
        tc.tile_pool(name="wop", bufs=2) as wop,
        tc.tile_pool(name="wf1p", bufs=3) as wf1p,
        tc.tile_pool(name="wf1lp", bufs=3) as wf1lp,
        tc.tile_pool(name="wf2p", bufs=2) as wf2p,
        tc.tile_pool(name="wf2lp", bufs=2) as wf2lp,
        tc.tile_pool(name="sm", bufs=1) as sm,
        tc.tile_pool(name="psum", bufs=8, space="PSUM") as pp,
        tc.tile_pool(name="dram", bufs=1, space="DRAM") as dram,
    ):
        # ---- constants ----
        ones32 = cpool.tile([32, 1], F32)
        nc.vector.memset(ones32[:], 1.0)
        ones128h = cpool.tile([128, 1], F16)
        nc.vector.memset(ones128h[:], 1.0)
        ones_r32 = cpool.tile([1, 32], F32)
        nc.vector.memset(ones_r32[:], 1.0)
        ones_r128 = cpool.tile([1, 128], F32)
        nc.vector.memset(ones_r128[:], 1.0)
        eighth = cpool.tile([1, 1], F32)
        nc.vector.memset(eighth[:], 1.0 / N_CORES)
        eps11 = cpool.tile([1, 1], F32)
        nc.vector.memset(eps11[:], 1e-6)
        ident1 = cpool.tile([1, 1], F32)
        nc.vector.memset(ident1[:], 1.0)
        ident1h = cpool.tile([1, 1], F16)
        nc.vector.memset(ident1h[:], 1.0)
        ident8 = cpool.tile([1, 1], F8E5)
        nc.vector.memset(ident8[:], 1.0)
        ident32 = cpool.tile([32, 32], F32)
        nc.sync.dma_start(ident32[:], i["ident32"])

        # sin/cos as rows (for k RoPE) and columns (for qT RoPE, q-scaled)
        sin_row = cpool.tile([1, 64], F32)
        cos_row = cpool.tile([1, 64], F32)
        nc.sync.dma_start(sin_row[:], i["sin"].rearrange("(a d) -> a d", a=1))
        nc.sync.dma_start(cos_row[:], i["cos"].rearrange("(a d) -> a d", a=1))
        sinq_row = cpool.tile([1, 64], F32)
        cosq_row = cpool.tile([1, 64], F32)
        nc.vector.tensor_scalar_mul(sinq_row[:], sin_row[:], SCALE)
        nc.vector.tensor_scalar_mul(cosq_row[:], cos_row[:], SCALE)

        # ---- rmsnorm -> cols [128, 32] f32 ----
        def rmsnorm_cols(x_dram, norm_dram, tag):
            x_rows = sm.tile([32, 128], F32, name=f"x_rows_{tag}", tag="x_rows")
            nrm_rows = sm.tile([32, 128], F32, name=f"nrm_rows_{tag}", tag="nrm_rows")
            nc.sync.dma_start(x_rows[:], x_dram.rearrange("(a d) -> a d", a=32))
            nc.sync.dma_start(nrm_rows[:], norm_dram.rearrange("(a d) -> a d", a=32))
            sq = sm.tile([32, 128], F32, name=f"sq_{tag}", tag="sq")
            ssq = sm.tile([32, 1], F32, name=f"ssq_{tag}", tag="ssq")
            nc.scalar.activation(sq[:], x_rows[:], AF.Square, accum_out=ssq[:])
            ms_psum = pp.tile([1, 1], F32, name=f"ms_psum_{tag}", tag="ps")
            nc.tensor.matmul(ms_psum[:], ones32[:], ssq[:])
            rstd = sm.tile([1, 1], F32, name=f"rstd_{tag}", tag="rstd")
            nc.scalar.activation(rstd[:], ms_psum[:], AF.Sqrt,
                                 bias=eps11[:], scale=1.0 / HIDDEN)
            nc.vector.reciprocal(rstd[:], rstd[:])
            rstd_ps = pp.tile([32, 1], F32, name=f"rstd_ps_{tag}", tag="ps")
            nc.tensor.matmul(rstd_ps[:], ones_r32[:], rstd[:])
            rstd32 = sm.tile([32, 1], F32, name=f"rstd32_{tag}", tag="rstd32")
            nc.vector.tensor_copy(rstd32[:], rstd_ps[:])
            h_rows = sm.tile([32, 128], F32, name=f"h_rows_{tag}", tag="h_rows")
            nc.vector.tensor_tensor(h_rows[:], x_rows[:], nrm_rows[:], ALU.mult)
            nc.vector.tensor_scalar_mul(h_rows[:], h_rows[:], rstd32[:])
            h_psum = pp.tile([128, 32], F32, name=f"h_psum_{tag}", tag="ps")
            nc.tensor.transpose(h_psum[:], h_rows[:], ident32[:])
            h_cols = sm.tile([128, 32], F32, name=f"h_cols_{tag}", tag="hcols")
            nc.vector.tensor_copy(h_cols[:], h_psum[:])
            return h_cols

        h_cols = rmsnorm_cols(i["x"], i["attn_norm"], "a")
        h16 = sm.tile([128, 32], F16, name="h16")
        nc.vector.tensor_copy(h16[:], h_cols[:])

        # ---- fused q/k/v rows: h @ [wq|wk|wv], weights moving ----
        q_ps = pp.tile([1, QKV_N], F32, name="q_ps", tag="ps")
        k_ps = pp.tile([1, QKV_N], F32, name="k_ps", tag="ps")
        v_ps = pp.tile([1, QKV_N], F32, name="v_ps", tag="ps")
        for t8 in range(8):
            wkv_t = wkvp.tile([128, 4, 3 * QKV_N], F16, name="wkv_t", tag="wkv")
            nc.sync.dma_start(wkv_t[:], i["wqkv"][t8])
            for b in range(4):
                kb = t8 * 4 + b
                nc.tensor.matmul(q_ps[:], h16[:, kb:kb + 1],
                                 wkv_t[:, b, 0:QKV_N],
                                 start=(kb == 0), stop=(kb == KB - 1))
                nc.tensor.matmul(k_ps[:], h16[:, kb:kb + 1],
                                 wkv_t[:, b, QKV_N:2 * QKV_N],
                                 start=(kb == 0), stop=(kb == KB - 1))
                nc.tensor.matmul(v_ps[:], h16[:, kb:kb + 1],
                                 wkv_t[:, b, 2 * QKV_N:3 * QKV_N],
                                 start=(kb == 0), stop=(kb == KB - 1))

        def rope_row(src_ps, cos_t, sin_t, tag):
            out = sm.tile([1, QKV_N], F32, name=f"rope_{tag}")
            tmp = sm.tile([1, QKV_N], F32, name=f"rope_tmp_{tag}")
            r3 = src_ps[:].rearrange("a (h d) -> a h d", h=HEADS_PC)
            o3 = out[:].rearrange("a (h d) -> a h d", h=HEADS_PC)
            t3 = tmp[:].rearrange("a (h d) -> a h d", h=HEADS_PC)
            cb = cos_t[:].unsqueeze(1).to_broadcast((1, HEADS_PC, 64))
            sb = sin_t[:].unsqueeze(1).to_broadcast((1, HEADS_PC, 64))
            nc.vector.tensor_tensor(o3[:, :, 0:64], r3[:, :, 0:64], cb, ALU.mult)
            nc.vector.tensor_tensor(t3[:, :, 0:64], r3[:, :, 64:128], sb, ALU.mult)
            nc.vector.tensor_sub(o3[:, :, 0:64], o3[:, :, 0:64], t3[:, :, 0:64])
            nc.vector.tensor_tensor(o3[:, :, 64:128], r3[:, :, 64:128], cb, ALU.mult)
            nc.vector.tensor_tensor(t3[:, :, 64:128], r3[:, :, 0:64], sb, ALU.mult)
            nc.vector.tensor_add(o3[:, :, 64:128], o3[:, :, 64:128],
                                 t3[:, :, 64:128])
            return out

        q_rot = rope_row(q_ps, cosq_row, sinq_row, "q")  # pre-scaled
        qrep_ps = pp.tile([128, QKV_N], F32, name="qrep_ps", tag="ps")
        nc.tensor.matmul(qrep_ps[:], ones_r128[:], q_rot[:])
        q_rep16 = sm.tile([128, QKV_N], F16, name="q_rep16")
        nc.vector.tensor_copy(q_rep16[:], qrep_ps[:])

        # ---- attention: DVE scores; AV via 97-col stationary so the 4
        # heads' outputs land on aligned PSUM rows 0/32/64/96 ----
        oacc = pp.tile([97, 512], F32, name="oacc", tag="ps")
        den_acc = sm.tile([1, HEADS_PC], F32, name="den_acc")
        nc.vector.memset(den_acc[:], 0.0)

        for g in range(4):
            k_sup = kp.tile([128, 8, QKV_N], F16, name="k_sup", tag="k")
            nc.sync.dma_start(k_sup[:], i["kc"][g])
            v_sup = vp.tile([128, 8, QKV_N], F16, name="v_sup", tag="v")
            nc.sync.dma_start(v_sup[:], i["vc"][g])
            s_f32 = sm.tile([128, 32], F32, name=f"s_f32_{g}", tag=f"sf{g % 2}")
            qb = q_rep16[:].unsqueeze(1).to_broadcast((128, 2, QKV_N))
            for qtr in range(4):
                scratch = sm.tile([128, 2, QKV_N], F32, name=f"scr_{g}_{qtr}",
                                  tag="scr")
                nc.vector.tensor_tensor(scratch[:],
                                        k_sup[:, qtr * 2:(qtr + 1) * 2, :],
                                        qb, ALU.mult)
                nc.vector.tensor_reduce(
                    s_f32[:, qtr * 8:(qtr + 1) * 8].rearrange(
                        "p (t h) -> p t h", h=HEADS_PC),
                    scratch[:].rearrange("p t (h d) -> p t h d", h=HEADS_PC),
                    mybir.AxisListType.X, ALU.add)
            exp_c = sm.tile([128, 32], F16, name=f"exp_{g}", tag=f"exp{g % 2}")
            nc.scalar.activation(exp_c[:], s_f32[:], AF.Exp)
            s_av = sm.tile([128, 8, 128], F16, name=f"s_av_{g}",
                           tag=f"sav{g % 2}")
            nc.vector.memset(s_av[:], 0.0)
            sav_view = s_av[:].rearrange("p t (h j) -> p t h j", j=32)
            nc.vector.tensor_copy(
                sav_view[:, :, :, 0:1],
                exp_c[:].rearrange("p (t h j) -> p t h j", h=HEADS_PC, j=1))
            den_ps = pp.tile([1, 32], F32, name="den_ps", tag="ps")
            nc.tensor.matmul(den_ps[:], ones128h[:], exp_c[:])
            den_g = sm.tile([1, HEADS_PC], F32, name="den_g", tag="deng")
            nc.vector.tensor_reduce(
                den_g[:],
                den_ps[:].rearrange("a (t h) -> a h t", h=HEADS_PC),
                mybir.AxisListType.X, ALU.add)
            nc.vector.tensor_add(den_acc[:], den_acc[:], den_g[:])
            for tt in range(8):
                nc.tensor.matmul(
                    oacc[:],
                    s_av[:, tt, 0:97],
                    v_sup[:, tt, :],
                    start=(g == 0 and tt == 0),
                    stop=(g == 3 and tt == 7),
                    skip_group_check=True,
                )

        # ---- current-token contribution (on rows) ----
        k_rot = rope_row(k_ps, cos_row, sin_row, "k")  # unscaled
        v16_row = sm.tile([1, QKV_N], F16, name="v16_row")
        nc.vector.tensor_copy(v16_row[:], v_ps[:])

        scr_new = sm.tile([1, QKV_N], F32, name="scr_new")
        nc.vector.tensor_tensor(scr_new[:], q_rot[:], k_rot[:], ALU.mult)
        s_new = sm.tile([1, HEADS_PC], F32, name="s_new")
        nc.vector.tensor_reduce(
            s_new[:],
            scr_new[:].rearrange("a (h d) -> a h d", h=HEADS_PC),
            mybir.AxisListType.X, ALU.add)
        e_new = sm.tile([1, HEADS_PC], F32, name="e_new")
        nc.scalar.activation(e_new[:], s_new[:], AF.Exp)
        nc.vector.tensor_add(den_acc[:], den_acc[:], e_new[:])

        # o row = (sum_t exp*v + e_new*v_new) / den, then transpose to cols
        o_row = sm.tile([1, QKV_N], F32, name="o_row_att")
        o3v = o_row[:].rearrange("a (h d) -> a h d", h=HEADS_PC)
        for h in range(HEADS_PC):
            nc.vector.tensor_copy(o_row[:, h * 128:(h + 1) * 128],
                                  oacc[32 * h:32 * h + 1,
                                       h * 128:(h + 1) * 128])
        vnew_sc = sm.tile([1, QKV_N], F32, name="vnew_sc")
        v3 = vnew_sc[:].rearrange("a (h d) -> a h d", h=HEADS_PC)
        eb = e_new[:].unsqueeze(2).to_broadcast((1, HEADS_PC, 128))
        nc.vector.tensor_tensor(v3[:], v_ps[:].rearrange(
            "a (h d) -> a h d", h=HEADS_PC), eb, ALU.mult)
        nc.vector.tensor_add(o_row[:], o_row[:], vnew_sc[:])
        nc.vector.reciprocal(den_acc[:], den_acc[:])
        rb = den_acc[:].unsqueeze(2).to_broadcast((1, HEADS_PC, 128))
        nc.vector.tensor_tensor(o3v[:], o3v[:], rb, ALU.mult)

        oT_ps = pp.tile([128, HEADS_PC], F32, name="oT_ps", tag="ps")
        for h in range(HEADS_PC):
            nc.tensor.transpose(oT_ps[:, h:h + 1],
                                o_row[:, h * 128:(h + 1) * 128], ident1[:])
        o_sb = sm.tile([128, HEADS_PC], F16, name="o_sb")
        nc.vector.tensor_copy(o_sb[:], oT_ps[:])

        # ---- o @ w_o + x/8 -> [1,4096] -> AllReduce #1 ----
        ar1_in = dram.tile([HIDDEN], F32, name="ar1_in")
        ar1_out = dram.tile([HIDDEN], F32, name="ar1_out")

        chunks1 = [pp.tile([1, 512], F32, name=f"c1_{n}", tag="ps")
                   for n in range(8)]
        for kb in range(HEADS_PC):
            wo_t = wop.tile([128, HIDDEN], F16, name="wo_t", tag="wo")
            nc.sync.dma_start(wo_t[:], i["wo"][kb * 128:(kb + 1) * 128, :])
            for n in range(8):
                nc.tensor.matmul(
                    chunks1[n][:], o_sb[:, kb:kb + 1],
                    wo_t[:, n * 512:(n + 1) * 512],
                    start=(kb == 0), stop=False,
                )
        for n in range(8):
            xch = sm.tile([1, 512], F32, name=f"xr_{n}", tag=f"xr{n % 2}")
            nc.sync.dma_start(
                xch[:], i["x"][n * 512:(n + 1) * 512].rearrange("(a d) -> a d", a=1))
            nc.tensor.matmul(
                chunks1[n][:], eighth[:], xch[:],
                start=False, stop=True,
            )
            orow_c = sm.tile([1, 512], F32, name=f"or_{n}", tag=f"or{n % 2}")
            nc.vector.tensor_copy(orow_c[:], chunks1[n][:])
            nc.sync.dma_start(ar1_in[n * 512:(n + 1) * 512], orow_c[:])
        nc.gpsimd.collective_compute(
            "AllReduce", ALU.add,
            replica_groups=[list(range(N_CORES))],
            ins=[ar1_in[:].opt()], outs=[ar1_out[:].opt()],
        )

        # ---- MLP ----
        h2_cols = rmsnorm_cols(ar1_out[:], i["ffn_norm"], "b")

        # h2 hi/lo fp16; s1[kb] = [h2h | 0*31 | h2l] stationaries (M=33);
        # h2hs8 = fp8e5m2(h2h / LO_SCALE) pairs with the scaled fp8 lo weights
        h2h = sm.tile([128, 32], F16, name="h2h")
        nc.vector.tensor_copy(h2h[:], h2_cols[:])
        h2h32 = sm.tile([128, 32], F32, name="h2h32")
        nc.vector.tensor_copy(h2h32[:], h2h[:])
        h2hs8 = sm.tile([128, 32], F8E5, name="h2hs8")
        nc.vector.tensor_scalar_mul(h2hs8[:], h2h32[:], 1.0 / LO_SCALE)
        nc.vector.tensor_sub(h2h32[:], h2_cols[:], h2h32[:])
        s1 = sm.tile([128, 32, 33], F16, name="s1")
        nc.vector.memset(s1[:], 0.0)
        h2c3 = h2h[:].rearrange("p (k j) -> p k j", j=1)
        l2c3 = h2h32[:].rearrange("p (k j) -> p k j", j=1)
        nc.vector.tensor_copy(s1[:, :, 0:1], h2c3)
        nc.vector.tensor_copy(s1[:, :, 32:33], l2c3)

        # wf1: h2-stationary (M=33: hi-part row 0, lo-part row 32), w moving
        FF1_CH = [(0, 512), (512, 1024), (1024, 1376)]
        pre_ps = [pp.tile([33, c1 - c0], F32, name=f"pre_{ci}", tag="ps")
                  for ci, (c0, c1) in enumerate(FF1_CH)]
        for t8 in range(8):
            w1h_t = wf1p.tile([128, 4, FF_N], F16, name="w1h_t", tag="wf1")
            nc.sync.dma_start(w1h_t[:], i["wf1h"][t8])
            w1l_t = wf1lp.tile([128, 4, FF_N], F8E5, name="w1l_t", tag="wf1l")
            nc.sync.dma_start(w1l_t[:], i["wf1l"][t8])
            for b in range(4):
                kb = t8 * 4 + b
                for ci, (c0, c1) in enumerate(FF1_CH):
                    nc.tensor.matmul(
                        pre_ps[ci][:],
                        s1[:, kb, :],
                        w1h_t[:, b, c0:c1],
                        start=(kb == 0), stop=False,
                        skip_group_check=True,
                    )
                    nc.tensor.matmul(
                        pre_ps[ci][0:1, :],
                        h2hs8[:, kb:kb + 1],
                        w1l_t[:, b, c0:c1],
                        start=False, stop=(kb == KB - 1),
                        skip_group_check=True,
                    )

        # pre = row0 + row32; silu on the row; a -> hi/lo rows
        pre_row = sm.tile([1, FF_N], F32, name="pre_row")
        for ci, (c0, c1) in enumerate(FF1_CH):
            pc = sm.tile([1, 512], F32, name=f"pc_{ci}", tag=f"pc{ci % 2}")
            nc.vector.tensor_copy(pc[:, 0:c1 - c0], pre_ps[ci][32:33, :])
            nc.vector.tensor_copy(pre_row[:, c0:c1], pre_ps[ci][0:1, :])
            nc.vector.tensor_tensor(pre_row[:, c0:c1], pre_row[:, c0:c1],
                                    pc[:, 0:c1 - c0], ALU.add)
        sig_row = sm.tile([1, FF_N], F32, name="sig_row", tag="row32b")
        nc.scalar.activation(sig_row[:], pre_row[:], AF.Sigmoid)
        a_row = pre_row  # in-place: a = pre * sigmoid(pre)
        nc.vector.tensor_tensor(a_row[:], pre_row[:], sig_row[:], ALU.mult)
        ah_row = sm.tile([1, FF_N], F16, name="ah_row")
        nc.vector.tensor_copy(ah_row[:], a_row[:])
        ah32_row = sm.tile([1, FF_N], F32, name="ah32_row", tag="row32b")
        nc.vector.tensor_copy(ah32_row[:], ah_row[:])
        aS_row = sm.tile([1, FF_N], F8E5, name="aS_row")
        nc.vector.tensor_scalar_mul(aS_row[:], ah32_row[:], 1.0 / LO_SCALE)
        nc.vector.tensor_sub(ah32_row[:], a_row[:], ah32_row[:])
        al_row = sm.tile([1, FF_N], F16, name="al_row")
        nc.vector.tensor_copy(al_row[:], ah32_row[:])

        # transpose a rows to columns (even cols: 4B-aligned PSUM writes);
        # build s2[kb] = [a_hi | 0*31 | a_lo] fp16 and s2s = fp8 scaled-hi
        aT_ps = pp.tile([128, 44], F16, name="aT_ps", tag="ps")
        aTs_ps = pp.tile([128, 44], F8E5, name="aTs_ps", tag="ps")
        for kb in range(11):
            sz = FF_KB_SIZES[kb]
            nc.tensor.transpose(aT_ps[0:sz, 2 * kb:2 * kb + 1],
                                ah_row[:, kb * 128:kb * 128 + sz], ident1h[:])
            nc.tensor.transpose(aT_ps[0:sz, 22 + 2 * kb:23 + 2 * kb],
                                al_row[:, kb * 128:kb * 128 + sz], ident1h[:])
            nc.tensor.transpose(aTs_ps[0:sz, 4 * kb:4 * kb + 1],
                                aS_row[:, kb * 128:kb * 128 + sz], ident8[:])
        s2 = sm.tile([128, 11, 33], F16, name="s2")
        nc.vector.memset(s2[:], 0.0)
        aTh3 = aT_ps[:, 0:22].rearrange("p (k j) -> p k j", j=2)
        aTl3 = aT_ps[:, 22:44].rearrange("p (k j) -> p k j", j=2)
        nc.vector.tensor_copy(s2[:, :, 0:1], aTh3[:, :, 0:1])
        nc.vector.tensor_copy(s2[:, :, 32:33], aTl3[:, :, 0:1])
        s2s = sm.tile([128, 11], F8E5, name="s2s")
        aTs3 = aTs_ps[:].rearrange("p (k j) -> p k j", j=4)
        s2s3 = s2s[:].rearrange("p (k j) -> p k j", j=1)
        nc.vector.tensor_copy(s2s3, aTs3[:, :, 0:1])

        # wf2: a-stationary (M=33), weights moving, two passes
        chunks2 = [pp.tile([33, 512], F32, name=f"c2_{n}", tag="ps")
                   for n in range(8)]
        for kb in range(11):
            sz = FF_KB_SIZES[kb]
            w2h_t = wf2p.tile([128, HIDDEN], F16, name="w2h_t", tag="wf2")
            nc.sync.dma_start(
                w2h_t[0:sz, :], i["wf2h"][kb * 128:kb * 128 + sz, :])
            w2l_t = wf2lp.tile([128, HIDDEN], F8E5, name="w2l_t", tag="wf2l")
            nc.sync.dma_start(
                w2l_t[0:sz, :], i["wf2l"][kb * 128:kb * 128 + sz, :])
            for n in range(8):
                nc.tensor.matmul(
                    chunks2[n][:],
                    s2[0:sz, kb, :],
                    w2h_t[0:sz, n * 512:(n + 1) * 512],
                    start=(kb == 0), stop=False,
                    skip_group_check=True,
                )
                nc.tensor.matmul(
                    chunks2[n][0:1, :],
                    s2s[0:sz, kb:kb + 1],
                    w2l_t[0:sz, n * 512:(n + 1) * 512],
                    start=False, stop=False,
                    skip_group_check=True,
                )

        ar2_in = dram.tile([HIDDEN], F32, name="ar2_in")
        ar2_out = dram.tile([HIDDEN], F32, name="ar2_out")
        for n in range(8):
            x2ch = sm.tile([1, 512], F32, name=f"x2r_{n}", tag=f"xr{n % 2}")
            nc.sync.dma_start(
                x2ch[:],
                ar1_out[n * 512:(n + 1) * 512].rearrange("(a d) -> a d", a=1))
            nc.tensor.matmul(
                chunks2[n][0:1, :], eighth[:], x2ch[:],
                start=False, stop=True,
                skip_group_check=True,
            )
            c2sb = sm.tile([1, 512], F32, name=f"c2sb_{n}", tag=f"pc{n % 2}")
            nc.vector.tensor_copy(c2sb[:], chunks2[n][32:33, :])
            ffc = sm.tile([1, 512], F32, name=f"ff_{n}", tag=f"or{n % 2}")
            nc.vector.tensor_copy(ffc[:], chunks2[n][0:1, :])
            nc.vector.tensor_tensor(ffc[:], ffc[:], c2sb[:], ALU.add)
            nc.sync.dma_start(ar2_in[n * 512:(n + 1) * 512], ffc[:])
        nc.gpsimd.collective_compute(
            "AllReduce", ALU.add,
            replica_groups=[list(range(N_CORES))],
            ins=[ar2_in[:].opt()], outs=[ar2_out[:].opt()],
        )
        nc.sync.dma_start(y[:], ar2_out[:])


_BUILT = None


def _build():
    global _BUILT
    if _BUILT is None:
        nc = bacc.Bacc("TRN2", target_bir_lowering=False, debug=False,
                       num_devices=N_CORES)
        with tile.TileContext(nc) as tc:
            _emit(nc, tc)
        nc.compile()
        _BUILT = nc
    return _BUILT


def _shard(inputs):
    import ml_dtypes
    E5 = ml_dtypes.float8_e5m2

    f = lambda a: np.ascontiguousarray(np.asarray(a, dtype=np.float32))
    f16 = lambda a: np.ascontiguousarray(np.asarray(a, dtype=np.float16))

    def hilo(a):
        hi = np.asarray(a, dtype=np.float16)
        lo = np.asarray((a - hi.astype(np.float32)) * 4096.0, dtype=E5)
        return np.ascontiguousarray(hi), np.ascontiguousarray(lo)

    x = f(inputs["x"])
    attn_norm = f(inputs["attn_norm"])
    ffn_norm = f(inputs["ffn_norm"])
    pos = int(np.asarray(inputs["pos"]))
    sin = f(inputs["sin_cache"][pos])
    cos = f(inputs["cos_cache"][pos])
    wq, wk, wv = [np.asarray(inputs[k], np.float32) for k in ("w_q", "w_k", "w_v")]
    wo = np.asarray(inputs["w_o"], np.float32)
    wf1 = np.asarray(inputs["w_ff1"], np.float32)
    wf2 = np.asarray(inputs["w_ff2"], np.float32)
    kc = np.asarray(inputs["k_cache"], np.float32).reshape(KV_LEN, N_HEADS * HEAD_DIM)
    vc = np.asarray(inputs["v_cache"], np.float32).reshape(KV_LEN, N_HEADS * HEAD_DIM)

    in_maps = []
    for c in range(N_CORES):
        qs = slice(c * QKV_N, (c + 1) * QKV_N)
        fs = slice(c * FF_N, (c + 1) * FF_N)
        w1h, w1l = hilo(wf1[:, fs])
        w2h, w2l = hilo(wf2[fs, :])
        tile4 = lambda a, nb, b: np.ascontiguousarray(
            a.reshape(nb, b, 128, a.shape[1]).transpose(0, 2, 1, 3))
        in_maps.append({
            "x": x,
            "ident32": np.eye(32, dtype=np.float32),
            "attn_norm": attn_norm,
            "ffn_norm": ffn_norm,
            "sin": sin,
            "cos": cos,
            "wqkv": tile4(f16(np.concatenate(
                [wq[:, qs], wk[:, qs], wv[:, qs]], axis=1)), 8, 4),
            "kc": tile4(f16(kc[:, qs]), 4, 8),
            "vc": tile4(f16(vc[:, qs]), 4, 8),
            "wo": f16(wo[qs, :]),
            "wf1h": tile4(w1h, 8, 4),
            "wf1l": tile4(w1l, 8, 4),
            "wf2h": w2h,
            "wf2l": w2l,
        })
    return in_maps


def kernel(**inputs):
    nc = _build()
    in_maps = _shard(inputs)
    res = bass_utils.run_bass_kernel_spmd(
        nc, in_maps, core_ids=list(range(N_CORES)))
    return res.results[0]["y"]
